# revision 28
# baseline (speedup 1.0000x reference)
"""Trainium2 kernel for nn_CosSimRouter_pad_merge_learn_local.

Math notes (verified against the jax reference to ~3e-7 absmax):

* Both attention blocks in the expander have key-length 1, so softmax over
  the key axis is identically 1 and each MHA collapses to
  ``(x @ Wv.T + bv) @ Wo.T + bo`` — the q/k projections and the additive
  mask are dead code.
* The final merge softmax masks every (s, r) pair to -inf except r's own
  best-match row s = bm[r].  Only Lr <= 575 pairs ever contribute, so the
  heavy FFN runs on Lr tokens, not Ls*Lr.

Work split:

* Host (numpy, f32, mirrors the reference op-for-op): routing, the tiny
  selected-token path (a few [Ls,1024]x[1024,1024] GEMMs), the pre-FFN
  layernorm, and the final score/softmax/blend.
* Device (8 NeuronCores, SPMD): the two FFN GEMMs — 97% of the FLOPs.
  The 4096 hidden dim is sharded 512 per core (weights are split, the
  token batch is replicated), each core returns a partial z_c = gelu(t @
  W1_c.T + b1_c) @ W2_c.T and the host sums the 8 partials.  Token count
  is padded to a static 576 so one compiled NEFF serves any routing.
"""

import numpy as np

EMBED_DIM = 1024
HIDDEN_DIM = 4096
GAMMA = 0.02
TEMP = 0.05
PAD = 1
L_V = 576
N_CORES = 8
H_PER_CORE = HIDDEN_DIM // N_CORES  # 512
T_PAD = 576  # static token capacity (Lr <= 575 always)

_DEVICE_PROG = None  # cached compiled Bass program
LAST_RESULTS = None  # BassKernelResults of the most recent device run


# ---------------------------------------------------------------------------
# Host-side math (mirrors reference.py in float32 numpy)
# ---------------------------------------------------------------------------

def _cos(a, b):
    an = a / np.maximum(np.sqrt((a * a).sum(-1, keepdims=True)), np.float32(1e-8))
    bn = b / np.maximum(np.sqrt((b * b).sum(-1, keepdims=True)), np.float32(1e-8))
    return an @ bn.T


def _route(vision_feature, text_embed, attention_mask):
    L_v = vision_feature.shape[0]
    cs = np.where(attention_mask[None, :], _cos(vision_feature, text_embed),
                  np.float32(0.0))
    m = cs.max(-1) / np.float32(TEMP)
    e = np.exp(m - m.max())
    s = (e / e.sum()).astype(np.float32)
    order = np.argsort(-s, kind="stable")
    cum = np.cumsum(s[order])
    k = max(int((cum <= GAMMA).sum()), 1)
    sel = order[:k]
    g = int(round(L_v ** 0.5))
    coords = np.stack([sel // g, sel % g], axis=1)
    offs = np.array([[i, j] for i in range(-PAD, PAD + 1)
                     for j in range(-PAD, PAD + 1) if (i, j) != (0, 0)])
    pc = np.clip(coords[:, None, :] + offs[None, :, :], 0, g - 1)
    uniq = np.unique(pc[..., 0] * g + pc[..., 1])
    rem = np.setdiff1d(np.arange(L_v), uniq)
    return uniq, rem


def _ln(x, w, b):
    mu = x.mean(-1, keepdims=True)
    v = ((x - mu) ** 2).mean(-1, keepdims=True)
    return (x - mu) / np.sqrt(v + np.float32(1e-5)) * w + b


# ---------------------------------------------------------------------------
# Device program: per-core partial FFN over the padded token batch
# ---------------------------------------------------------------------------

MM_DTYPE = "bf16"  # "f32" or "bf16": dtype of the matmul operands


def _build_device_prog():
    import concourse.tile as tile
    from concourse import bacc, mybir

    nc = bacc.Bacc("TRN2", target_bir_lowering=False, debug=False,
                   num_devices=N_CORES)
    dt = mybir.dt.float32
    mmdt = mybir.dt.bfloat16 if MM_DTYPE == "bf16" else mybir.dt.float32
    D, H, T = EMBED_DIM, H_PER_CORE, T_PAD
    KD, KH = D // 128, H // 128  # 8, 4

    # Inputs arrive pre-tiled into the SBUF layout (partition dim first).
    t_in = nc.dram_tensor("t_t", [128, KD, T], mmdt, kind="ExternalInput").ap()
    w1_in = nc.dram_tensor("w1t", [128, KD, H], mmdt, kind="ExternalInput").ap()
    w2_in = nc.dram_tensor("w2t", [128, KH, D], mmdt, kind="ExternalInput").ap()
    b1_in = nc.dram_tensor("b1", [128, KH], dt, kind="ExternalInput").ap()
    z_out = nc.dram_tensor("z_t", [128, KD, T], mmdt, kind="ExternalOutput").ap()

    # One PSUM bank holds 512 fp32 per partition; split T into chunks.
    n_chunks = []
    n0 = 0
    while n0 < T:
        n_chunks.append((n0, min(512, T - n0)))
        n0 += 512

    with tile.TileContext(nc) as tc:
        with (
            tc.tile_pool(name="acts", bufs=1) as acts,
            tc.tile_pool(name="psum", bufs=8, space="PSUM") as psum,
        ):
            tt = acts.tile([128, KD, T], mmdt, tag="tt")
            w1 = acts.tile([128, KD, H], mmdt, tag="w1")
            w2 = acts.tile([128, KH, D], mmdt, tag="w2")
            b1 = acts.tile([128, KH], dt, tag="b1")
            ht = acts.tile([128, KH, T], mmdt, tag="ht")
            zt = acts.tile([128, KD, T], mmdt, tag="zt")

            # Split input DMAs along the contraction dim so the first
            # matmuls can start as soon as their chunks land.
            for kd in range(KD):
                nc.sync.dma_start(tt[:, kd], t_in[:, kd])
                nc.sync.dma_start(w1[:, kd], w1_in[:, kd])
            nc.sync.dma_start(b1[:], b1_in[:])
            for kh in range(KH):
                nc.sync.dma_start(w2[:, kh], w2_in[:, kh])

            # GEMM1 + gelu: ht[hp, n] = gelu(sum_kd w1[kd, hp*128:].T @ tt[kd, n])
            # NOTE: accumulation groups (start..stop) must not interleave —
            # interleaving two PSUM groups corrupts results on hardware.
            for hp in range(KH):
                for (nst, nlen) in n_chunks:
                    pt = psum.tile([128, 512], dt, tag="ps", name="ps")[:, :nlen]
                    for kd in range(KD):
                        nc.tensor.matmul(
                            pt,
                            lhsT=w1[:, kd, hp * 128:(hp + 1) * 128],
                            rhs=tt[:, kd, nst:nst + nlen],
                            start=(kd == 0), stop=(kd == KD - 1),
                        )
                    nc.scalar.activation(
                        ht[:, hp, nst:nst + nlen], pt,
                        mybir.ActivationFunctionType.Gelu,
                        bias=b1[:, hp:hp + 1],
                    )

            # GEMM2: z[dp, n] = sum_kh w2[kh, dp*128:].T @ ht[kh, n];
            # stage via SBUF (DMA cannot read PSUM) and stream each dp
            # slice to DRAM as soon as it is complete.
            for dp in range(KD):
                for (nst, nlen) in n_chunks:
                    pt = psum.tile([128, 512], dt, tag="ps", name="ps")[:, :nlen]
                    for kh in range(KH):
                        nc.tensor.matmul(
                            pt,
                            lhsT=w2[:, kh, dp * 128:(dp + 1) * 128],
                            rhs=ht[:, kh, nst:nst + nlen],
                            start=(kh == 0), stop=(kh == KH - 1),
                        )
                    nc.vector.tensor_copy(zt[:, dp, nst:nst + nlen], pt)
                nc.sync.dma_start(z_out[:, dp], zt[:, dp])

    nc.compile()
    return nc


def _get_device_prog():
    global _DEVICE_PROG
    if _DEVICE_PROG is None:
        _DEVICE_PROG = _build_device_prog()
    return _DEVICE_PROG


def _tile_pf(a):
    """[K*128, F] row-major -> [128, K, F] (partition dim first)."""
    k128, f = a.shape
    return np.ascontiguousarray(
        a.reshape(k128 // 128, 128, f).transpose(1, 0, 2))


def _device_ffn_z(t, params):
    """z = gelu(t @ f1_w.T + f1_b) @ f2_w.T for t [Lr, D], via 8 cores."""
    import os

    from concourse import bass_utils

    # This container lacks the axon NTFF hook module; a BASS_TRACE env var
    # would crash run_bass_kernel_spmd on import, so force tracing off.
    os.environ["BASS_NEVER_TRACE"] = "1"

    if MM_DTYPE == "bf16":
        import ml_dtypes
        mm_np = ml_dtypes.bfloat16
    else:
        mm_np = np.float32

    Lr = t.shape[0]
    t_pad = np.zeros((T_PAD, EMBED_DIM), mm_np)
    t_pad[:Lr] = t.astype(mm_np)
    tt = _tile_pf(np.ascontiguousarray(t_pad.T))  # [128, 8, 576]

    f1_w = np.asarray(params["f1_w"], np.float32).astype(mm_np)  # [4096, 1024]
    f1_b = np.asarray(params["f1_b"], np.float32)                # [4096]
    f2_w = np.asarray(params["f2_w"], np.float32).astype(mm_np)  # [1024, 4096]

    in_maps = []
    for c in range(N_CORES):
        hs = slice(c * H_PER_CORE, (c + 1) * H_PER_CORE)
        w1t = _tile_pf(np.ascontiguousarray(f1_w[hs].T))       # [128, 8, 512]
        w2t = _tile_pf(np.ascontiguousarray(f2_w[:, hs].T))    # [128, 4, 1024]
        b1 = np.ascontiguousarray(
            f1_b[hs].reshape(H_PER_CORE // 128, 128).T)        # [128, 4]
        in_maps.append({"t_t": tt, "w1t": w1t, "w2t": w2t, "b1": b1})

    nc = _get_device_prog()
    try:
        res = bass_utils.run_bass_kernel_spmd(
            nc, in_maps, core_ids=list(range(N_CORES)))
    except Exception:
        import time
        time.sleep(5)  # transient device errors: retry once
        res = bass_utils.run_bass_kernel_spmd(
            nc, in_maps, core_ids=list(range(N_CORES)))
    global LAST_RESULTS
    LAST_RESULTS = res
    z = np.zeros((T_PAD, EMBED_DIM), np.float32)
    for r in res.results:
        zt = np.asarray(r["z_t"], np.float32)  # [128, 8, 576]
        z += zt.transpose(1, 0, 2).reshape(EMBED_DIM, T_PAD).T
    return z[:Lr]


# ---------------------------------------------------------------------------
# Top-level kernel
# ---------------------------------------------------------------------------

def kernel(vision_feature, text_embed, attention_mask, params):
    vf = np.asarray(vision_feature, np.float32)
    te = np.asarray(text_embed, np.float32)
    am = np.asarray(attention_mask)
    p = {k: np.asarray(v, np.float32) for k, v in params.items()}

    uniq, rem = _route(vf, te, am)
    Ls = len(uniq)
    sel_feat = vf[uniq]
    rem_feat = vf[rem]
    Lr = rem_feat.shape[0]

    # Selected-token path (degenerate MHAs -> linear projections).
    wv_sa = p["sa_wqkv"][2 * EMBED_DIM:]
    bv_sa = p["sa_bqkv"][2 * EMBED_DIM:]
    x1 = (sel_feat @ wv_sa.T + bv_sa) @ p["sa_wo"].T + p["sa_bo"]
    x1 = _ln(x1 + sel_feat, p["n1_w"], p["n1_b"])
    wv_ca = p["ca_wqkv"][2 * EMBED_DIM:]
    bv_ca = p["ca_bqkv"][2 * EMBED_DIM:]
    ca = (x1 @ wv_ca.T + bv_ca) @ p["ca_wo"].T + p["ca_bo"]  # [Ls, D]

    out = sel_feat.copy()
    if Lr > 0:
        bm = np.argmax(_cos(rem_feat, sel_feat), axis=1)  # [Lr]
        t = _ln(ca[bm] + rem_feat, p["n2_w"], p["n2_b"]).astype(np.float32)

        try:
            z = _device_ffn_z(t, p)  # [Lr, D]
        except Exception:
            # Last-resort host fallback (device unavailable): exact math.
            from scipy.special import erf
            h = t @ p["f1_w"].T + p["f1_b"]
            h = h * 0.5 * (1.0 + erf(h / np.sqrt(2.0)))
            z = (h @ p["f2_w"].T).astype(np.float32)

        x3 = _ln(t + z + p["f2_b"], p["n4_w"], p["n4_b"])
        logit = x3 @ p["sp_w"][0] + p["sp_b"][0]
        score = 1.0 / (1.0 + np.exp(-logit))  # [Lr]

        for si in range(Ls):
            rs = np.where(bm == si)[0]
            if len(rs) == 0:
                continue
            vals = np.concatenate([np.ones(1, np.float32), score[rs]])
            w = np.exp(vals - vals.max())
            w /= w.sum()
            out[si] = w[0] * sel_feat[si] + w[1:] @ rem_feat[rs]

    return out.astype(np.float32)


# revision 52
# speedup vs baseline: 1.1803x; 1.1803x over previous
"""Trainium2 kernel for nn_CosSimRouter_pad_merge_learn_local.

Math notes (verified against the jax reference to ~3e-7 absmax):

* Both attention blocks in the expander have key-length 1, so softmax over
  the key axis is identically 1 and each MHA collapses to
  ``(x @ Wv.T + bv) @ Wo.T + bo`` — the q/k projections and the additive
  mask are dead code.
* The final merge softmax masks every (s, r) pair to -inf except r's own
  best-match row s = bm[r].  Only Lr <= 575 pairs ever contribute, so the
  heavy FFN runs on Lr tokens, not Ls*Lr.

Work split:

* Host (numpy, f32, mirrors the reference op-for-op): routing, the tiny
  selected-token path (a few [Ls,1024]x[1024,1024] GEMMs), the pre-FFN
  layernorm, and the final score/softmax/blend.
* Device (8 NeuronCores, SPMD): the two FFN GEMMs — 97% of the FLOPs.
  The 4096 hidden dim is sharded 512 per core (weights are split, the
  token batch is replicated), each core returns a partial z_c = gelu(t @
  W1_c.T + b1_c) @ W2_c.T and the host sums the 8 partials.  Token count
  is padded to a static 576 so one compiled NEFF serves any routing.
"""

import numpy as np

EMBED_DIM = 1024
HIDDEN_DIM = 4096
GAMMA = 0.02
TEMP = 0.05
PAD = 1
L_V = 576
N_CORES = 8
H_PER_CORE = HIDDEN_DIM // N_CORES  # 512
T_PAD = 576  # token capacity upper bound (Lr <= 575 always)

_DEVICE_PROGS = {}  # token-count -> compiled Bass program
LAST_RESULTS = None  # BassKernelResults of the most recent device run


# ---------------------------------------------------------------------------
# Host-side math (mirrors reference.py in float32 numpy)
# ---------------------------------------------------------------------------

def _cos(a, b):
    an = a / np.maximum(np.sqrt((a * a).sum(-1, keepdims=True)), np.float32(1e-8))
    bn = b / np.maximum(np.sqrt((b * b).sum(-1, keepdims=True)), np.float32(1e-8))
    return an @ bn.T


def _route(vision_feature, text_embed, attention_mask):
    L_v = vision_feature.shape[0]
    cs = np.where(attention_mask[None, :], _cos(vision_feature, text_embed),
                  np.float32(0.0))
    m = cs.max(-1) / np.float32(TEMP)
    e = np.exp(m - m.max())
    s = (e / e.sum()).astype(np.float32)
    order = np.argsort(-s, kind="stable")
    cum = np.cumsum(s[order])
    k = max(int((cum <= GAMMA).sum()), 1)
    sel = order[:k]
    g = int(round(L_v ** 0.5))
    coords = np.stack([sel // g, sel % g], axis=1)
    offs = np.array([[i, j] for i in range(-PAD, PAD + 1)
                     for j in range(-PAD, PAD + 1) if (i, j) != (0, 0)])
    pc = np.clip(coords[:, None, :] + offs[None, :, :], 0, g - 1)
    uniq = np.unique(pc[..., 0] * g + pc[..., 1])
    rem = np.setdiff1d(np.arange(L_v), uniq)
    return uniq, rem


def _ln(x, w, b):
    mu = x.mean(-1, keepdims=True)
    v = ((x - mu) ** 2).mean(-1, keepdims=True)
    return (x - mu) / np.sqrt(v + np.float32(1e-5)) * w + b


# ---------------------------------------------------------------------------
# Device program: per-core partial FFN over the padded token batch
# ---------------------------------------------------------------------------

MM_DTYPE = "bf16"  # "f32" or "bf16": dtype of the matmul operands


def _build_device_prog(T):
    import concourse.tile as tile
    from concourse import bacc, mybir

    nc = bacc.Bacc("TRN2", target_bir_lowering=False, debug=False,
                   num_devices=N_CORES)
    dt = mybir.dt.float32
    mmdt = mybir.dt.bfloat16 if MM_DTYPE == "bf16" else mybir.dt.float32
    D, H = EMBED_DIM, H_PER_CORE
    KD, KH = D // 128, H // 128  # 8, 4

    # Inputs arrive pre-tiled into the SBUF layout (partition dim first).
    t_in = nc.dram_tensor("t_t", [128, KD, T], mmdt, kind="ExternalInput").ap()
    w1_in = nc.dram_tensor("w1t", [128, KD, H], mmdt, kind="ExternalInput").ap()
    w2_in = nc.dram_tensor("w2t", [128, KH, D], mmdt, kind="ExternalInput").ap()
    b1_in = nc.dram_tensor("b1", [128, KH], dt, kind="ExternalInput").ap()
    z_out = nc.dram_tensor("z_t", [128, KD, T], mmdt, kind="ExternalOutput").ap()

    # One PSUM bank holds 512 fp32 per partition; split T into balanced
    # chunks (equal halves beat 512+tail in the timeline model).
    if T <= 512:
        n_chunks = [(0, T)]
    else:
        h1 = (T + 1) // 2
        n_chunks = [(0, h1), (h1, T - h1)]

    with tile.TileContext(nc) as tc:
        with (
            tc.tile_pool(name="acts", bufs=1) as acts,
            tc.tile_pool(name="psum", bufs=8, space="PSUM") as psum,
        ):
            tt = acts.tile([128, KD, T], mmdt, tag="tt")
            w1 = acts.tile([128, KD, H], mmdt, tag="w1")
            w2 = acts.tile([128, KH, D], mmdt, tag="w2")
            b1 = acts.tile([128, KH], dt, tag="b1")
            ht = acts.tile([128, KH, T], mmdt, tag="ht")
            zt = acts.tile([128, KD, T], mmdt, tag="zt")

            # Split input DMAs along the contraction dim so the first
            # matmuls can start as soon as their chunks land, and spread
            # them over BOTH HWDGE rings (SP=nc.sync, ACT=nc.scalar) —
            # each dma_start occupies its ring ~0.6us, so one ring
            # serializes ~29 DMAs into the whole kernel span.
            nc.sync.dma_start(w1[:, 0], w1_in[:, 0])
            nc.sync.dma_start(tt[:, 0], t_in[:, 0])
            nc.sync.dma_start(w1[:, 1], w1_in[:, 1])
            nc.sync.dma_start(tt[:, 1], t_in[:, 1])
            for kd in range(2, KD, 2):
                nc.sync.dma_start(w1[:, kd:kd + 2], w1_in[:, kd:kd + 2])
                nc.sync.dma_start(tt[:, kd:kd + 2], t_in[:, kd:kd + 2])
            nc.sync.dma_start(b1[:], b1_in[:])
            for kh in range(KH):
                nc.sync.dma_start(w2[:, kh], w2_in[:, kh])

            # GEMM1 + gelu: ht[hp, n] = gelu(sum_kd w1[kd, hp*128:].T @ tt[kd, n])
            # NOTE: accumulation groups (start..stop) must not interleave —
            # interleaving two PSUM groups corrupts results on hardware.
            # Chunk-outer order: chunk 0's four hp groups (and gelus) finish
            # while chunk 1 is still on the PE, so GEMM2 never waits on gelu.
            for (nst, nlen) in n_chunks:
                for hp in range(KH):
                    pt = psum.tile([128, 512], dt, tag="ps", name="ps")[:, :nlen]
                    for kd in range(KD):
                        nc.tensor.matmul(
                            pt,
                            lhsT=w1[:, kd, hp * 128:(hp + 1) * 128],
                            rhs=tt[:, kd, nst:nst + nlen],
                            start=(kd == 0), stop=(kd == KD - 1),
                        )
                    nc.scalar.activation(
                        ht[:, hp, nst:nst + nlen], pt,
                        mybir.ActivationFunctionType.Gelu,
                        bias=b1[:, hp:hp + 1],
                    )

            # GEMM2: z[dp, n] = sum_kh w2[kh, dp*128:].T @ ht[kh, n];
            # stage via SBUF (DMA cannot read PSUM) and stream each dp
            # slice to DRAM as soon as it is complete.
            for dp in range(KD):
                for (nst, nlen) in n_chunks:
                    pt = psum.tile([128, 512], dt, tag="ps", name="ps")[:, :nlen]
                    for kh in range(KH):
                        nc.tensor.matmul(
                            pt,
                            lhsT=w2[:, kh, dp * 128:(dp + 1) * 128],
                            rhs=ht[:, kh, nst:nst + nlen],
                            start=(kh == 0), stop=(kh == KH - 1),
                        )
                    nc.vector.tensor_copy(zt[:, dp, nst:nst + nlen], pt)
                nc.sync.dma_start(z_out[:, dp], zt[:, dp])

    nc.compile()
    return nc


def _get_device_prog(T=T_PAD):
    if T not in _DEVICE_PROGS:
        _DEVICE_PROGS[T] = _build_device_prog(T)
    return _DEVICE_PROGS[T]


def _tile_pf(a):
    """[K*128, F] row-major -> [128, K, F] (partition dim first)."""
    k128, f = a.shape
    return np.ascontiguousarray(
        a.reshape(k128 // 128, 128, f).transpose(1, 0, 2))


def _device_ffn_z(t, params):
    """z = gelu(t @ f1_w.T + f1_b) @ f2_w.T for t [Lr, D], via 8 cores."""
    import os

    from concourse import bass_utils

    # This container lacks the axon NTFF hook module; a BASS_TRACE env var
    # would crash run_bass_kernel_spmd on import, so force tracing off.
    os.environ["BASS_NEVER_TRACE"] = "1"

    if MM_DTYPE == "bf16":
        import ml_dtypes
        mm_np = ml_dtypes.bfloat16
    else:
        mm_np = np.float32

    # Size the program to the actual token count (rounded to keep DMA rows
    # 4B-aligned); a new count recompiles, the usual count hits the cache.
    Lr = t.shape[0]
    T_used = min(T_PAD, Lr + (-Lr) % 2)
    t_pad = np.zeros((T_used, EMBED_DIM), mm_np)
    t_pad[:Lr] = t.astype(mm_np)
    tt = _tile_pf(np.ascontiguousarray(t_pad.T))  # [128, 8, T_used]

    f1_w = np.asarray(params["f1_w"], np.float32).astype(mm_np)  # [4096, 1024]
    f1_b = np.asarray(params["f1_b"], np.float32)                # [4096]
    f2_w = np.asarray(params["f2_w"], np.float32).astype(mm_np)  # [1024, 4096]

    in_maps = []
    for c in range(N_CORES):
        hs = slice(c * H_PER_CORE, (c + 1) * H_PER_CORE)
        w1t = _tile_pf(np.ascontiguousarray(f1_w[hs].T))       # [128, 8, 512]
        w2t = _tile_pf(np.ascontiguousarray(f2_w[:, hs].T))    # [128, 4, 1024]
        b1 = np.ascontiguousarray(
            f1_b[hs].reshape(H_PER_CORE // 128, 128).T)        # [128, 4]
        in_maps.append({"t_t": tt, "w1t": w1t, "w2t": w2t, "b1": b1})

    nc = _get_device_prog(T_used)
    try:
        res = bass_utils.run_bass_kernel_spmd(
            nc, in_maps, core_ids=list(range(N_CORES)))
    except Exception:
        import time
        time.sleep(5)  # transient device errors: retry once
        res = bass_utils.run_bass_kernel_spmd(
            nc, in_maps, core_ids=list(range(N_CORES)))
    global LAST_RESULTS
    LAST_RESULTS = res
    z = np.zeros((T_used, EMBED_DIM), np.float32)
    for r in res.results:
        zt = np.asarray(r["z_t"], np.float32)  # [128, 8, T_used]
        z += zt.transpose(1, 0, 2).reshape(EMBED_DIM, T_used).T
    return z[:Lr]


# ---------------------------------------------------------------------------
# Top-level kernel
# ---------------------------------------------------------------------------

def kernel(vision_feature, text_embed, attention_mask, params):
    vf = np.asarray(vision_feature, np.float32)
    te = np.asarray(text_embed, np.float32)
    am = np.asarray(attention_mask)
    p = {k: np.asarray(v, np.float32) for k, v in params.items()}

    uniq, rem = _route(vf, te, am)
    Ls = len(uniq)
    sel_feat = vf[uniq]
    rem_feat = vf[rem]
    Lr = rem_feat.shape[0]

    # Selected-token path (degenerate MHAs -> linear projections).
    wv_sa = p["sa_wqkv"][2 * EMBED_DIM:]
    bv_sa = p["sa_bqkv"][2 * EMBED_DIM:]
    x1 = (sel_feat @ wv_sa.T + bv_sa) @ p["sa_wo"].T + p["sa_bo"]
    x1 = _ln(x1 + sel_feat, p["n1_w"], p["n1_b"])
    wv_ca = p["ca_wqkv"][2 * EMBED_DIM:]
    bv_ca = p["ca_bqkv"][2 * EMBED_DIM:]
    ca = (x1 @ wv_ca.T + bv_ca) @ p["ca_wo"].T + p["ca_bo"]  # [Ls, D]

    out = sel_feat.copy()
    if Lr > 0:
        bm = np.argmax(_cos(rem_feat, sel_feat), axis=1)  # [Lr]
        t = _ln(ca[bm] + rem_feat, p["n2_w"], p["n2_b"]).astype(np.float32)

        try:
            z = _device_ffn_z(t, p)  # [Lr, D]
        except Exception:
            # Last-resort host fallback (device unavailable): exact math.
            from scipy.special import erf
            h = t @ p["f1_w"].T + p["f1_b"]
            h = h * 0.5 * (1.0 + erf(h / np.sqrt(2.0)))
            z = (h @ p["f2_w"].T).astype(np.float32)

        x3 = _ln(t + z + p["f2_b"], p["n4_w"], p["n4_b"])
        logit = x3 @ p["sp_w"][0] + p["sp_b"][0]
        score = 1.0 / (1.0 + np.exp(-logit))  # [Lr]

        for si in range(Ls):
            rs = np.where(bm == si)[0]
            if len(rs) == 0:
                continue
            vals = np.concatenate([np.ones(1, np.float32), score[rs]])
            w = np.exp(vals - vals.max())
            w /= w.sum()
            out[si] = w[0] * sel_feat[si] + w[1:] @ rem_feat[rs]

    return out.astype(np.float32)


# revision 59
# speedup vs baseline: 1.2079x; 1.0233x over previous
"""Trainium2 kernel for nn_CosSimRouter_pad_merge_learn_local.

Math notes (verified against the jax reference to ~3e-7 absmax):

* Both attention blocks in the expander have key-length 1, so softmax over
  the key axis is identically 1 and each MHA collapses to
  ``(x @ Wv.T + bv) @ Wo.T + bo`` — the q/k projections and the additive
  mask are dead code.
* The final merge softmax masks every (s, r) pair to -inf except r's own
  best-match row s = bm[r].  Only Lr <= 575 pairs ever contribute, so the
  heavy FFN runs on Lr tokens, not Ls*Lr.

Work split:

* Host (numpy, f32, mirrors the reference op-for-op): routing, the tiny
  selected-token path (a few [Ls,1024]x[1024,1024] GEMMs), the pre-FFN
  layernorm, and the final score/softmax/blend.
* Device (8 NeuronCores, SPMD): the two FFN GEMMs — 97% of the FLOPs.
  The 4096 hidden dim is sharded 512 per core (weights are split, the
  token batch is replicated), each core returns a partial z_c = gelu(t @
  W1_c.T + b1_c) @ W2_c.T and the host sums the 8 partials.  The program
  is built for the actual token count (cached per count, padded to even).
"""

import numpy as np

EMBED_DIM = 1024
HIDDEN_DIM = 4096
GAMMA = 0.02
TEMP = 0.05
PAD = 1
L_V = 576
N_CORES = 8
H_PER_CORE = HIDDEN_DIM // N_CORES  # 512
T_PAD = 576  # token capacity upper bound (Lr <= 575 always)

_DEVICE_PROGS = {}  # token-count -> compiled Bass program
LAST_RESULTS = None  # BassKernelResults of the most recent device run
LAST_T = T_PAD  # token count of the most recent device run


# ---------------------------------------------------------------------------
# Host-side math (mirrors reference.py in float32 numpy)
# ---------------------------------------------------------------------------

def _cos(a, b):
    an = a / np.maximum(np.sqrt((a * a).sum(-1, keepdims=True)), np.float32(1e-8))
    bn = b / np.maximum(np.sqrt((b * b).sum(-1, keepdims=True)), np.float32(1e-8))
    return an @ bn.T


def _route(vision_feature, text_embed, attention_mask):
    L_v = vision_feature.shape[0]
    cs = np.where(attention_mask[None, :], _cos(vision_feature, text_embed),
                  np.float32(0.0))
    m = cs.max(-1) / np.float32(TEMP)
    e = np.exp(m - m.max())
    s = (e / e.sum()).astype(np.float32)
    order = np.argsort(-s, kind="stable")
    cum = np.cumsum(s[order])
    k = max(int((cum <= GAMMA).sum()), 1)
    sel = order[:k]
    g = int(round(L_v ** 0.5))
    coords = np.stack([sel // g, sel % g], axis=1)
    offs = np.array([[i, j] for i in range(-PAD, PAD + 1)
                     for j in range(-PAD, PAD + 1) if (i, j) != (0, 0)])
    pc = np.clip(coords[:, None, :] + offs[None, :, :], 0, g - 1)
    uniq = np.unique(pc[..., 0] * g + pc[..., 1])
    rem = np.setdiff1d(np.arange(L_v), uniq)
    return uniq, rem


def _ln(x, w, b):
    mu = x.mean(-1, keepdims=True)
    v = ((x - mu) ** 2).mean(-1, keepdims=True)
    return (x - mu) / np.sqrt(v + np.float32(1e-5)) * w + b


# ---------------------------------------------------------------------------
# Device program: per-core partial FFN over the padded token batch
# ---------------------------------------------------------------------------

MM_DTYPE = "bf16"  # "f32" or "bf16": dtype of the matmul operands


def _build_device_prog(T):
    import concourse.tile as tile
    from concourse import bacc, mybir

    nc = bacc.Bacc("TRN2", target_bir_lowering=False, debug=False,
                   num_devices=N_CORES)
    dt = mybir.dt.float32
    mmdt = mybir.dt.bfloat16 if MM_DTYPE == "bf16" else mybir.dt.float32
    D, H = EMBED_DIM, H_PER_CORE
    KD, KH = D // 128, H // 128  # 8, 4

    # Inputs arrive pre-tiled into the SBUF layout (partition dim first).
    t_in = nc.dram_tensor("t_t", [128, KD, T], mmdt, kind="ExternalInput").ap()
    w1_in = nc.dram_tensor("w1t", [128, KD, H], mmdt, kind="ExternalInput").ap()
    w2_in = nc.dram_tensor("w2t", [128, KH, D], mmdt, kind="ExternalInput").ap()
    b1_in = nc.dram_tensor("b1", [128, KH], dt, kind="ExternalInput").ap()
    z_out = nc.dram_tensor("z_t", [128, KD, T], mmdt, kind="ExternalOutput").ap()

    # One PSUM bank holds 512 fp32 per partition; split T into balanced
    # chunks (equal halves beat 512+tail in the timeline model).
    if T <= 512:
        n_chunks = [(0, T)]
    else:
        h1 = (T + 1) // 2
        n_chunks = [(0, h1), (h1, T - h1)]

    with tile.TileContext(nc) as tc:
        with (
            tc.tile_pool(name="acts", bufs=1) as acts,
            tc.tile_pool(name="psum", bufs=8, space="PSUM") as psum,
        ):
            tt = acts.tile([128, KD, T], mmdt, tag="tt")
            w1 = acts.tile([128, KD, H], mmdt, tag="w1")
            w2 = acts.tile([128, KH, D], mmdt, tag="w2")
            b1 = acts.tile([128, KH], dt, tag="b1")
            ht = acts.tile([128, KH, T], mmdt, tag="ht")
            zt = acts.tile([128, KD, T], mmdt, tag="zt")

            # Input DMA schedule (tuned on the cost-model timeline): two
            # single-kd chunks first so the first matmul group starts
            # ~1.2us earlier, then kd-pairs — pairs amortize the ~0.6us
            # per-DMA HWDGE ring occupancy while keeping the PE fed.
            nc.sync.dma_start(w1[:, 0], w1_in[:, 0])
            nc.sync.dma_start(tt[:, 0], t_in[:, 0])
            nc.sync.dma_start(w1[:, 1], w1_in[:, 1])
            nc.sync.dma_start(tt[:, 1], t_in[:, 1])
            for kd in range(2, KD, 2):
                nc.sync.dma_start(w1[:, kd:kd + 2], w1_in[:, kd:kd + 2])
                nc.sync.dma_start(tt[:, kd:kd + 2], t_in[:, kd:kd + 2])
            nc.sync.dma_start(b1[:], b1_in[:])
            for kh in range(KH):
                nc.sync.dma_start(w2[:, kh], w2_in[:, kh])

            # GEMM1 + gelu: ht[hp, n] = gelu(sum_kd w1[kd, hp*128:].T @ tt[kd, n])
            # NOTE: accumulation groups (start..stop) must not interleave —
            # interleaving two PSUM groups corrupts results on hardware.
            # Chunk-outer order: chunk 0's four hp groups (and gelus) finish
            # while chunk 1 is still on the PE, so GEMM2 never waits on gelu.
            for (nst, nlen) in n_chunks:
                for hp in range(KH):
                    pt = psum.tile([128, 512], dt, tag="ps", name="ps")[:, :nlen]
                    for kd in range(KD):
                        nc.tensor.matmul(
                            pt,
                            lhsT=w1[:, kd, hp * 128:(hp + 1) * 128],
                            rhs=tt[:, kd, nst:nst + nlen],
                            start=(kd == 0), stop=(kd == KD - 1),
                        )
                    nc.scalar.activation(
                        ht[:, hp, nst:nst + nlen], pt,
                        mybir.ActivationFunctionType.Gelu,
                        bias=b1[:, hp:hp + 1],
                    )

            # GEMM2: z[dp, n] = sum_kh w2[kh, dp*128:].T @ ht[kh, n];
            # stage via SBUF (DMA cannot read PSUM) and stream each dp
            # slice to DRAM as soon as it is complete.
            for dp in range(KD):
                for (nst, nlen) in n_chunks:
                    pt = psum.tile([128, 512], dt, tag="ps", name="ps")[:, :nlen]
                    for kh in range(KH):
                        nc.tensor.matmul(
                            pt,
                            lhsT=w2[:, kh, dp * 128:(dp + 1) * 128],
                            rhs=ht[:, kh, nst:nst + nlen],
                            start=(kh == 0), stop=(kh == KH - 1),
                        )
                    nc.vector.tensor_copy(zt[:, dp, nst:nst + nlen], pt)
                nc.sync.dma_start(z_out[:, dp], zt[:, dp])

    nc.compile()
    return nc


def _get_device_prog(T=T_PAD):
    if T not in _DEVICE_PROGS:
        _DEVICE_PROGS[T] = _build_device_prog(T)
    return _DEVICE_PROGS[T]


def _tile_pf(a):
    """[K*128, F] row-major -> [128, K, F] (partition dim first)."""
    k128, f = a.shape
    return np.ascontiguousarray(
        a.reshape(k128 // 128, 128, f).transpose(1, 0, 2))


def _device_ffn_z(t, params):
    """z = gelu(t @ f1_w.T + f1_b) @ f2_w.T for t [Lr, D], via 8 cores."""
    import os

    from concourse import bass_utils

    # This container lacks the axon NTFF hook module; a BASS_TRACE env var
    # would crash run_bass_kernel_spmd on import, so force tracing off.
    os.environ["BASS_NEVER_TRACE"] = "1"

    if MM_DTYPE == "bf16":
        import ml_dtypes
        mm_np = ml_dtypes.bfloat16
    else:
        mm_np = np.float32

    # Size the program to the actual token count (rounded to keep DMA rows
    # 4B-aligned); a new count recompiles, the usual count hits the cache.
    Lr = t.shape[0]
    T_used = min(T_PAD, Lr + (-Lr) % 2)
    t_pad = np.zeros((T_used, EMBED_DIM), mm_np)
    t_pad[:Lr] = t.astype(mm_np)
    tt = _tile_pf(np.ascontiguousarray(t_pad.T))  # [128, 8, T_used]

    f1_w = np.asarray(params["f1_w"], np.float32).astype(mm_np)  # [4096, 1024]
    f1_b = np.asarray(params["f1_b"], np.float32)                # [4096]
    f2_w = np.asarray(params["f2_w"], np.float32).astype(mm_np)  # [1024, 4096]

    in_maps = []
    for c in range(N_CORES):
        hs = slice(c * H_PER_CORE, (c + 1) * H_PER_CORE)
        w1t = _tile_pf(np.ascontiguousarray(f1_w[hs].T))       # [128, 8, 512]
        w2t = _tile_pf(np.ascontiguousarray(f2_w[:, hs].T))    # [128, 4, 1024]
        b1 = np.ascontiguousarray(
            f1_b[hs].reshape(H_PER_CORE // 128, 128).T)        # [128, 4]
        in_maps.append({"t_t": tt, "w1t": w1t, "w2t": w2t, "b1": b1})

    global LAST_T
    LAST_T = T_used
    nc = _get_device_prog(T_used)
    try:
        res = bass_utils.run_bass_kernel_spmd(
            nc, in_maps, core_ids=list(range(N_CORES)))
    except Exception:
        import time
        time.sleep(5)  # transient device errors: retry once
        res = bass_utils.run_bass_kernel_spmd(
            nc, in_maps, core_ids=list(range(N_CORES)))
    global LAST_RESULTS
    LAST_RESULTS = res
    z = np.zeros((T_used, EMBED_DIM), np.float32)
    for r in res.results:
        zt = np.asarray(r["z_t"], np.float32)  # [128, 8, T_used]
        z += zt.transpose(1, 0, 2).reshape(EMBED_DIM, T_used).T
    return z[:Lr]


# ---------------------------------------------------------------------------
# Top-level kernel
# ---------------------------------------------------------------------------

def kernel(vision_feature, text_embed, attention_mask, params):
    vf = np.asarray(vision_feature, np.float32)
    te = np.asarray(text_embed, np.float32)
    am = np.asarray(attention_mask)
    p = {k: np.asarray(v, np.float32) for k, v in params.items()}

    uniq, rem = _route(vf, te, am)
    Ls = len(uniq)
    sel_feat = vf[uniq]
    rem_feat = vf[rem]
    Lr = rem_feat.shape[0]

    # Selected-token path (degenerate MHAs -> linear projections).
    wv_sa = p["sa_wqkv"][2 * EMBED_DIM:]
    bv_sa = p["sa_bqkv"][2 * EMBED_DIM:]
    x1 = (sel_feat @ wv_sa.T + bv_sa) @ p["sa_wo"].T + p["sa_bo"]
    x1 = _ln(x1 + sel_feat, p["n1_w"], p["n1_b"])
    wv_ca = p["ca_wqkv"][2 * EMBED_DIM:]
    bv_ca = p["ca_bqkv"][2 * EMBED_DIM:]
    ca = (x1 @ wv_ca.T + bv_ca) @ p["ca_wo"].T + p["ca_bo"]  # [Ls, D]

    out = sel_feat.copy()
    if Lr > 0:
        bm = np.argmax(_cos(rem_feat, sel_feat), axis=1)  # [Lr]
        t = _ln(ca[bm] + rem_feat, p["n2_w"], p["n2_b"]).astype(np.float32)

        try:
            z = _device_ffn_z(t, p)  # [Lr, D]
        except Exception:
            # Last-resort host fallback (device unavailable): exact math.
            from scipy.special import erf
            h = t @ p["f1_w"].T + p["f1_b"]
            h = h * 0.5 * (1.0 + erf(h / np.sqrt(2.0)))
            z = (h @ p["f2_w"].T).astype(np.float32)

        x3 = _ln(t + z + p["f2_b"], p["n4_w"], p["n4_b"])
        logit = x3 @ p["sp_w"][0] + p["sp_b"][0]
        score = 1.0 / (1.0 + np.exp(-logit))  # [Lr]

        for si in range(Ls):
            rs = np.where(bm == si)[0]
            if len(rs) == 0:
                continue
            vals = np.concatenate([np.ones(1, np.float32), score[rs]])
            w = np.exp(vals - vals.max())
            w /= w.sum()
            out[si] = w[0] * sel_feat[si] + w[1:] @ rem_feat[rs]

    return out.astype(np.float32)


# revision 61
# speedup vs baseline: 1.2355x; 1.0229x over previous
"""Trainium2 kernel for nn_CosSimRouter_pad_merge_learn_local.

Math notes (verified against the jax reference to ~3e-7 absmax):

* Both attention blocks in the expander have key-length 1, so softmax over
  the key axis is identically 1 and each MHA collapses to
  ``(x @ Wv.T + bv) @ Wo.T + bo`` — the q/k projections and the additive
  mask are dead code.
* The final merge softmax masks every (s, r) pair to -inf except r's own
  best-match row s = bm[r].  Only Lr <= 575 pairs ever contribute, so the
  heavy FFN runs on Lr tokens, not Ls*Lr.

Work split:

* Host (numpy, f32, mirrors the reference op-for-op): routing, the tiny
  selected-token path (a few [Ls,1024]x[1024,1024] GEMMs), the pre-FFN
  layernorm, and the final score/softmax/blend.
* Device (8 NeuronCores, SPMD): the two FFN GEMMs — 97% of the FLOPs.
  The 4096 hidden dim is sharded 512 per core (weights are split, the
  token batch is replicated), each core returns a partial z_c = gelu(t @
  W1_c.T + b1_c) @ W2_c.T and the host sums the 8 partials.  The program
  is built for the actual token count (cached per count, padded to even).
"""

import numpy as np

EMBED_DIM = 1024
HIDDEN_DIM = 4096
GAMMA = 0.02
TEMP = 0.05
PAD = 1
L_V = 576
N_CORES = 8
H_PER_CORE = HIDDEN_DIM // N_CORES  # 512
T_PAD = 576  # token capacity upper bound (Lr <= 575 always)

_DEVICE_PROGS = {}  # token-count -> compiled Bass program
LAST_RESULTS = None  # BassKernelResults of the most recent device run
LAST_T = T_PAD  # token count of the most recent device run


# ---------------------------------------------------------------------------
# Host-side math (mirrors reference.py in float32 numpy)
# ---------------------------------------------------------------------------

def _cos(a, b):
    an = a / np.maximum(np.sqrt((a * a).sum(-1, keepdims=True)), np.float32(1e-8))
    bn = b / np.maximum(np.sqrt((b * b).sum(-1, keepdims=True)), np.float32(1e-8))
    return an @ bn.T


def _route(vision_feature, text_embed, attention_mask):
    L_v = vision_feature.shape[0]
    cs = np.where(attention_mask[None, :], _cos(vision_feature, text_embed),
                  np.float32(0.0))
    m = cs.max(-1) / np.float32(TEMP)
    e = np.exp(m - m.max())
    s = (e / e.sum()).astype(np.float32)
    order = np.argsort(-s, kind="stable")
    cum = np.cumsum(s[order])
    k = max(int((cum <= GAMMA).sum()), 1)
    sel = order[:k]
    g = int(round(L_v ** 0.5))
    coords = np.stack([sel // g, sel % g], axis=1)
    offs = np.array([[i, j] for i in range(-PAD, PAD + 1)
                     for j in range(-PAD, PAD + 1) if (i, j) != (0, 0)])
    pc = np.clip(coords[:, None, :] + offs[None, :, :], 0, g - 1)
    uniq = np.unique(pc[..., 0] * g + pc[..., 1])
    rem = np.setdiff1d(np.arange(L_v), uniq)
    return uniq, rem


def _ln(x, w, b):
    mu = x.mean(-1, keepdims=True)
    v = ((x - mu) ** 2).mean(-1, keepdims=True)
    return (x - mu) / np.sqrt(v + np.float32(1e-5)) * w + b


# ---------------------------------------------------------------------------
# Device program: per-core partial FFN over the padded token batch
# ---------------------------------------------------------------------------

MM_DTYPE = "bf16"  # "f32" or "bf16": dtype of the matmul operands


def _build_device_prog(T):
    import concourse.tile as tile
    from concourse import bacc, mybir

    nc = bacc.Bacc("TRN2", target_bir_lowering=False, debug=False,
                   num_devices=N_CORES)
    dt = mybir.dt.float32
    mmdt = mybir.dt.bfloat16 if MM_DTYPE == "bf16" else mybir.dt.float32
    D, H = EMBED_DIM, H_PER_CORE
    KD, KH = D // 128, H // 128  # 8, 4

    # Inputs arrive pre-tiled into the SBUF layout (partition dim first).
    t_in = nc.dram_tensor("t_t", [128, KD, T], mmdt, kind="ExternalInput").ap()
    w1_in = nc.dram_tensor("w1t", [128, KD, H], mmdt, kind="ExternalInput").ap()
    w2_in = nc.dram_tensor("w2t", [128, KH, D], mmdt, kind="ExternalInput").ap()
    b1_in = nc.dram_tensor("b1", [128, KH], dt, kind="ExternalInput").ap()
    z_out = nc.dram_tensor("z_t", [128, KD, T], mmdt, kind="ExternalOutput").ap()

    # One PSUM bank holds 512 fp32 per partition; split T into balanced
    # chunks (equal halves beat 512+tail in the timeline model).
    if T <= 512:
        n_chunks = [(0, T)]
    else:
        h1 = (T + 1) // 2
        n_chunks = [(0, h1), (h1, T - h1)]

    with tile.TileContext(nc) as tc:
        with (
            tc.tile_pool(name="acts", bufs=1) as acts,
            tc.tile_pool(name="psum", bufs=8, space="PSUM") as psum,
        ):
            tt = acts.tile([128, KD, T], mmdt, tag="tt")
            w1 = acts.tile([128, KD, H], mmdt, tag="w1")
            w2 = acts.tile([128, KH, D], mmdt, tag="w2")
            b1 = acts.tile([128, KH], dt, tag="b1")
            ht = acts.tile([128, KH, T], mmdt, tag="ht")
            zt = acts.tile([128, KD, T], mmdt, tag="zt")

            # Input DMA schedule (tuned on the cost-model timeline): two
            # single-kd chunks first so the first matmul group starts
            # ~1.2us earlier, then kd-pairs — pairs amortize the ~0.6us
            # per-DMA HWDGE ring occupancy while keeping the PE fed.
            nc.sync.dma_start(w1[:, 0], w1_in[:, 0])
            nc.sync.dma_start(tt[:, 0], t_in[:, 0])
            nc.sync.dma_start(w1[:, 1], w1_in[:, 1])
            nc.sync.dma_start(tt[:, 1], t_in[:, 1])
            for kd in range(2, KD, 2):
                nc.sync.dma_start(w1[:, kd:kd + 2], w1_in[:, kd:kd + 2])
                nc.sync.dma_start(tt[:, kd:kd + 2], t_in[:, kd:kd + 2])
            nc.sync.dma_start(b1[:], b1_in[:])
            for kh in range(KH):
                nc.sync.dma_start(w2[:, kh], w2_in[:, kh])

            # GEMM1 + gelu: ht[hp, n] = gelu(sum_kd w1[kd, hp*128:].T @ tt[kd, n])
            # NOTE: accumulation groups (start..stop) must not interleave —
            # interleaving two PSUM groups corrupts results on hardware.
            # Chunk-outer order: chunk 0's four hp groups (and gelus) finish
            # while chunk 1 is still on the PE, so GEMM2 never waits on gelu.
            for (nst, nlen) in n_chunks:
                for hp in range(KH):
                    pt = psum.tile([128, 512], dt, tag="ps", name="ps")[:, :nlen]
                    for kd in range(KD):
                        nc.tensor.matmul(
                            pt,
                            lhsT=w1[:, kd, hp * 128:(hp + 1) * 128],
                            rhs=tt[:, kd, nst:nst + nlen],
                            start=(kd == 0), stop=(kd == KD - 1),
                        )
                    nc.scalar.activation(
                        ht[:, hp, nst:nst + nlen], pt,
                        mybir.ActivationFunctionType.Gelu,
                        bias=b1[:, hp:hp + 1],
                    )

            # GEMM2: z[dp, n] = sum_kh w2[kh, dp*128:].T @ ht[kh, n];
            # stage via SBUF (DMA cannot read PSUM) and stream each dp
            # slice to DRAM as soon as it is complete.
            for dp in range(KD):
                for (nst, nlen) in n_chunks:
                    pt = psum.tile([128, 512], dt, tag="ps", name="ps")[:, :nlen]
                    for kh in range(KH):
                        nc.tensor.matmul(
                            pt,
                            lhsT=w2[:, kh, dp * 128:(dp + 1) * 128],
                            rhs=ht[:, kh, nst:nst + nlen],
                            start=(kh == 0), stop=(kh == KH - 1),
                        )
                    nc.vector.tensor_copy(zt[:, dp, nst:nst + nlen], pt)
                nc.sync.dma_start(z_out[:, dp], zt[:, dp])

    nc.compile()
    return nc


def _get_device_prog(T=T_PAD):
    if T not in _DEVICE_PROGS:
        _DEVICE_PROGS[T] = _build_device_prog(T)
    return _DEVICE_PROGS[T]


def _tile_pf(a):
    """[K*128, F] row-major -> [128, K, F] (partition dim first)."""
    k128, f = a.shape
    return np.ascontiguousarray(
        a.reshape(k128 // 128, 128, f).transpose(1, 0, 2))


def _device_ffn_z(t, params):
    """z = gelu(t @ f1_w.T + f1_b) @ f2_w.T for t [Lr, D], via 8 cores."""
    import os

    from concourse import bass_utils

    # This container lacks the axon NTFF hook module; a BASS_TRACE env var
    # would crash run_bass_kernel_spmd on import, so force tracing off.
    os.environ["BASS_NEVER_TRACE"] = "1"

    if MM_DTYPE == "bf16":
        import ml_dtypes
        mm_np = ml_dtypes.bfloat16
    else:
        mm_np = np.float32

    # Size the program to the actual token count (rounded to keep DMA rows
    # 4B-aligned); a new count recompiles, the usual count hits the cache.
    Lr = t.shape[0]
    T_used = min(T_PAD, Lr + (-Lr) % 2)
    t_pad = np.zeros((T_used, EMBED_DIM), mm_np)
    t_pad[:Lr] = t.astype(mm_np)
    tt = _tile_pf(np.ascontiguousarray(t_pad.T))  # [128, 8, T_used]

    f1_w = np.asarray(params["f1_w"], np.float32).astype(mm_np)  # [4096, 1024]
    f1_b = np.asarray(params["f1_b"], np.float32)                # [4096]
    f2_w = np.asarray(params["f2_w"], np.float32).astype(mm_np)  # [1024, 4096]

    in_maps = []
    for c in range(N_CORES):
        hs = slice(c * H_PER_CORE, (c + 1) * H_PER_CORE)
        w1t = _tile_pf(np.ascontiguousarray(f1_w[hs].T))       # [128, 8, 512]
        w2t = _tile_pf(np.ascontiguousarray(f2_w[:, hs].T))    # [128, 4, 1024]
        b1 = np.ascontiguousarray(
            f1_b[hs].reshape(H_PER_CORE // 128, 128).T)        # [128, 4]
        in_maps.append({"t_t": tt, "w1t": w1t, "w2t": w2t, "b1": b1})

    global LAST_T
    LAST_T = T_used
    nc = _get_device_prog(T_used)
    try:
        res = bass_utils.run_bass_kernel_spmd(
            nc, in_maps, core_ids=list(range(N_CORES)))
    except Exception:
        import time
        time.sleep(5)  # transient device errors: retry once
        res = bass_utils.run_bass_kernel_spmd(
            nc, in_maps, core_ids=list(range(N_CORES)))
    global LAST_RESULTS
    LAST_RESULTS = res
    z = np.zeros((T_used, EMBED_DIM), np.float32)
    for r in res.results:
        zt = np.asarray(r["z_t"], np.float32)  # [128, 8, T_used]
        z += zt.transpose(1, 0, 2).reshape(EMBED_DIM, T_used).T
    return z[:Lr]


# ---------------------------------------------------------------------------
# Top-level kernel
# ---------------------------------------------------------------------------

def kernel(vision_feature, text_embed, attention_mask, params):
    vf = np.asarray(vision_feature, np.float32)
    te = np.asarray(text_embed, np.float32)
    am = np.asarray(attention_mask)
    p = {k: np.asarray(v, np.float32) for k, v in params.items()}

    uniq, rem = _route(vf, te, am)
    Ls = len(uniq)
    sel_feat = vf[uniq]
    rem_feat = vf[rem]
    Lr = rem_feat.shape[0]

    # Selected-token path (degenerate MHAs -> linear projections).
    wv_sa = p["sa_wqkv"][2 * EMBED_DIM:]
    bv_sa = p["sa_bqkv"][2 * EMBED_DIM:]
    x1 = (sel_feat @ wv_sa.T + bv_sa) @ p["sa_wo"].T + p["sa_bo"]
    x1 = _ln(x1 + sel_feat, p["n1_w"], p["n1_b"])
    wv_ca = p["ca_wqkv"][2 * EMBED_DIM:]
    bv_ca = p["ca_bqkv"][2 * EMBED_DIM:]
    ca = (x1 @ wv_ca.T + bv_ca) @ p["ca_wo"].T + p["ca_bo"]  # [Ls, D]

    out = sel_feat.copy()
    if Lr > 0:
        bm = np.argmax(_cos(rem_feat, sel_feat), axis=1)  # [Lr]
        t = _ln(ca[bm] + rem_feat, p["n2_w"], p["n2_b"]).astype(np.float32)

        try:
            z = _device_ffn_z(t, p)  # [Lr, D]
        except Exception:
            # Last-resort host fallback (device unavailable): exact math.
            from scipy.special import erf
            h = t @ p["f1_w"].T + p["f1_b"]
            h = h * 0.5 * (1.0 + erf(h / np.sqrt(2.0)))
            z = (h @ p["f2_w"].T).astype(np.float32)

        x3 = _ln(t + z + p["f2_b"], p["n4_w"], p["n4_b"])
        logit = x3 @ p["sp_w"][0] + p["sp_b"][0]
        score = 1.0 / (1.0 + np.exp(-logit))  # [Lr]

        for si in range(Ls):
            rs = np.where(bm == si)[0]
            if len(rs) == 0:
                continue
            vals = np.concatenate([np.ones(1, np.float32), score[rs]])
            w = np.exp(vals - vals.max())
            w /= w.sum()
            out[si] = w[0] * sel_feat[si] + w[1:] @ rem_feat[rs]

    return out.astype(np.float32)


# revision 63
# speedup vs baseline: 1.2589x; 1.0189x over previous
"""Trainium2 kernel for nn_CosSimRouter_pad_merge_learn_local.

Math notes (verified against the jax reference to ~3e-7 absmax):

* Both attention blocks in the expander have key-length 1, so softmax over
  the key axis is identically 1 and each MHA collapses to
  ``(x @ Wv.T + bv) @ Wo.T + bo`` — the q/k projections and the additive
  mask are dead code.
* The final merge softmax masks every (s, r) pair to -inf except r's own
  best-match row s = bm[r].  Only Lr <= 575 pairs ever contribute, so the
  heavy FFN runs on Lr tokens, not Ls*Lr.

Work split:

* Host (numpy, f32, mirrors the reference op-for-op): routing, the tiny
  selected-token path (a few [Ls,1024]x[1024,1024] GEMMs), the pre-FFN
  layernorm, and the final score/softmax/blend.
* Device (8 NeuronCores, SPMD): the two FFN GEMMs — 97% of the FLOPs.
  The 4096 hidden dim is sharded 512 per core (weights are split, the
  token batch is replicated), each core returns a partial z_c = gelu(t @
  W1_c.T + b1_c) @ W2_c.T and the host sums the 8 partials.  The program
  is built for the actual token count (cached per count, padded to even).
"""

import numpy as np

EMBED_DIM = 1024
HIDDEN_DIM = 4096
GAMMA = 0.02
TEMP = 0.05
PAD = 1
L_V = 576
N_CORES = 8
H_PER_CORE = HIDDEN_DIM // N_CORES  # 512
T_PAD = 576  # token capacity upper bound (Lr <= 575 always)

_DEVICE_PROGS = {}  # token-count -> compiled Bass program
LAST_RESULTS = None  # BassKernelResults of the most recent device run
LAST_T = T_PAD  # token count of the most recent device run


# ---------------------------------------------------------------------------
# Host-side math (mirrors reference.py in float32 numpy)
# ---------------------------------------------------------------------------

def _cos(a, b):
    an = a / np.maximum(np.sqrt((a * a).sum(-1, keepdims=True)), np.float32(1e-8))
    bn = b / np.maximum(np.sqrt((b * b).sum(-1, keepdims=True)), np.float32(1e-8))
    return an @ bn.T


def _route(vision_feature, text_embed, attention_mask):
    L_v = vision_feature.shape[0]
    cs = np.where(attention_mask[None, :], _cos(vision_feature, text_embed),
                  np.float32(0.0))
    m = cs.max(-1) / np.float32(TEMP)
    e = np.exp(m - m.max())
    s = (e / e.sum()).astype(np.float32)
    order = np.argsort(-s, kind="stable")
    cum = np.cumsum(s[order])
    k = max(int((cum <= GAMMA).sum()), 1)
    sel = order[:k]
    g = int(round(L_v ** 0.5))
    coords = np.stack([sel // g, sel % g], axis=1)
    offs = np.array([[i, j] for i in range(-PAD, PAD + 1)
                     for j in range(-PAD, PAD + 1) if (i, j) != (0, 0)])
    pc = np.clip(coords[:, None, :] + offs[None, :, :], 0, g - 1)
    uniq = np.unique(pc[..., 0] * g + pc[..., 1])
    rem = np.setdiff1d(np.arange(L_v), uniq)
    return uniq, rem


def _ln(x, w, b):
    mu = x.mean(-1, keepdims=True)
    v = ((x - mu) ** 2).mean(-1, keepdims=True)
    return (x - mu) / np.sqrt(v + np.float32(1e-5)) * w + b


# ---------------------------------------------------------------------------
# Device program: per-core partial FFN over the padded token batch
# ---------------------------------------------------------------------------

MM_DTYPE = "bf16"  # "f32" or "bf16": dtype of the matmul operands


def _build_device_prog(T):
    import concourse.tile as tile
    from concourse import bacc, mybir

    nc = bacc.Bacc("TRN2", target_bir_lowering=False, debug=False,
                   num_devices=N_CORES)
    dt = mybir.dt.float32
    mmdt = mybir.dt.bfloat16 if MM_DTYPE == "bf16" else mybir.dt.float32
    D, H = EMBED_DIM, H_PER_CORE
    KD, KH = D // 128, H // 128  # 8, 4

    # Inputs arrive pre-tiled into the SBUF layout (partition dim first).
    t_in = nc.dram_tensor("t_t", [128, KD, T], mmdt, kind="ExternalInput").ap()
    w1_in = nc.dram_tensor("w1t", [128, KD, H], mmdt, kind="ExternalInput").ap()
    w2_in = nc.dram_tensor("w2t", [128, KH, D], mmdt, kind="ExternalInput").ap()
    b1_in = nc.dram_tensor("b1", [128, KH], dt, kind="ExternalInput").ap()
    z_out = nc.dram_tensor("z_t", [128, KD, T], mmdt, kind="ExternalOutput").ap()

    # One PSUM bank holds 512 fp32 per partition; split T into balanced
    # chunks (equal halves beat 512+tail in the timeline model).
    if T <= 512:
        n_chunks = [(0, T)]
    else:
        h1 = (T + 1) // 2
        n_chunks = [(0, h1), (h1, T - h1)]

    with tile.TileContext(nc) as tc:
        with (
            tc.tile_pool(name="acts", bufs=1) as acts,
            tc.tile_pool(name="psum", bufs=8, space="PSUM") as psum,
        ):
            tt = acts.tile([128, KD, T], mmdt, tag="tt")
            w1 = acts.tile([128, KD, H], mmdt, tag="w1")
            w2 = acts.tile([128, KH, D], mmdt, tag="w2")
            b1 = acts.tile([128, KH], dt, tag="b1")
            ht = acts.tile([128, KH, T], mmdt, tag="ht")
            zt = acts.tile([128, KD, T], mmdt, tag="zt")

            # Input DMA schedule (tuned on the cost-model timeline): two
            # single-kd chunks first so the first matmul group starts
            # ~1.2us earlier, then kd-pairs — pairs amortize the ~0.6us
            # per-DMA HWDGE ring occupancy while keeping the PE fed.
            nc.sync.dma_start(w1[:, 0], w1_in[:, 0])
            nc.sync.dma_start(tt[:, 0], t_in[:, 0])
            nc.sync.dma_start(w1[:, 1], w1_in[:, 1])
            nc.sync.dma_start(tt[:, 1], t_in[:, 1])
            for kd in range(2, KD, 2):
                nc.sync.dma_start(w1[:, kd:kd + 2], w1_in[:, kd:kd + 2])
                nc.sync.dma_start(tt[:, kd:kd + 2], t_in[:, kd:kd + 2])
            nc.sync.dma_start(b1[:], b1_in[:])
            for kh in range(KH):
                nc.sync.dma_start(w2[:, kh], w2_in[:, kh])

            # GEMM1 + gelu: ht[hp, n] = gelu(sum_kd w1[kd, hp*128:].T @ tt[kd, n])
            # NOTE: accumulation groups (start..stop) must not interleave —
            # interleaving two PSUM groups corrupts results on hardware.
            # Chunk-outer order: chunk 0's four hp groups (and gelus) finish
            # while chunk 1 is still on the PE, so GEMM2 never waits on gelu.
            for (nst, nlen) in n_chunks:
                for hp in range(KH):
                    pt = psum.tile([128, 512], dt, tag="ps", name="ps")[:, :nlen]
                    for kd in range(KD):
                        nc.tensor.matmul(
                            pt,
                            lhsT=w1[:, kd, hp * 128:(hp + 1) * 128],
                            rhs=tt[:, kd, nst:nst + nlen],
                            start=(kd == 0), stop=(kd == KD - 1),
                        )
                    nc.scalar.activation(
                        ht[:, hp, nst:nst + nlen], pt,
                        mybir.ActivationFunctionType.Gelu,
                        bias=b1[:, hp:hp + 1],
                    )

            # GEMM2: z[dp, n] = sum_kh w2[kh, dp*128:].T @ ht[kh, n];
            # stage via SBUF (DMA cannot read PSUM) and stream each dp
            # slice to DRAM as soon as it is complete.
            for dp in range(KD):
                for (nst, nlen) in n_chunks:
                    pt = psum.tile([128, 512], dt, tag="ps", name="ps")[:, :nlen]
                    for kh in range(KH):
                        nc.tensor.matmul(
                            pt,
                            lhsT=w2[:, kh, dp * 128:(dp + 1) * 128],
                            rhs=ht[:, kh, nst:nst + nlen],
                            start=(kh == 0), stop=(kh == KH - 1),
                        )
                    nc.vector.tensor_copy(zt[:, dp, nst:nst + nlen], pt)
                nc.sync.dma_start(z_out[:, dp], zt[:, dp])

    nc.compile()
    return nc


def _get_device_prog(T=T_PAD):
    if T not in _DEVICE_PROGS:
        _DEVICE_PROGS[T] = _build_device_prog(T)
    return _DEVICE_PROGS[T]


def _tile_pf(a):
    """[K*128, F] row-major -> [128, K, F] (partition dim first)."""
    k128, f = a.shape
    return np.ascontiguousarray(
        a.reshape(k128 // 128, 128, f).transpose(1, 0, 2))


def _device_ffn_z(t, params):
    """z = gelu(t @ f1_w.T + f1_b) @ f2_w.T for t [Lr, D], via 8 cores."""
    import os

    from concourse import bass_utils

    # This container lacks the axon NTFF hook module; a BASS_TRACE env var
    # would crash run_bass_kernel_spmd on import, so force tracing off.
    os.environ["BASS_NEVER_TRACE"] = "1"

    if MM_DTYPE == "bf16":
        import ml_dtypes
        mm_np = ml_dtypes.bfloat16
    else:
        mm_np = np.float32

    # Size the program to the actual token count (rounded to keep DMA rows
    # 4B-aligned); a new count recompiles, the usual count hits the cache.
    Lr = t.shape[0]
    T_used = min(T_PAD, Lr + (-Lr) % 2)
    t_pad = np.zeros((T_used, EMBED_DIM), mm_np)
    t_pad[:Lr] = t.astype(mm_np)
    tt = _tile_pf(np.ascontiguousarray(t_pad.T))  # [128, 8, T_used]

    f1_w = np.asarray(params["f1_w"], np.float32).astype(mm_np)  # [4096, 1024]
    f1_b = np.asarray(params["f1_b"], np.float32)                # [4096]
    f2_w = np.asarray(params["f2_w"], np.float32).astype(mm_np)  # [1024, 4096]

    in_maps = []
    for c in range(N_CORES):
        hs = slice(c * H_PER_CORE, (c + 1) * H_PER_CORE)
        w1t = _tile_pf(np.ascontiguousarray(f1_w[hs].T))       # [128, 8, 512]
        w2t = _tile_pf(np.ascontiguousarray(f2_w[:, hs].T))    # [128, 4, 1024]
        b1 = np.ascontiguousarray(
            f1_b[hs].reshape(H_PER_CORE // 128, 128).T)        # [128, 4]
        in_maps.append({"t_t": tt, "w1t": w1t, "w2t": w2t, "b1": b1})

    global LAST_T
    LAST_T = T_used
    nc = _get_device_prog(T_used)
    res = None
    for attempt in range(3):
        try:
            res = bass_utils.run_bass_kernel_spmd(
                nc, in_maps, core_ids=list(range(N_CORES)))
            break
        except Exception as e:
            import sys
            import time
            print(f"device run attempt {attempt + 1} failed: {e!r}",
                  file=sys.stderr)
            if attempt == 2:
                raise
            time.sleep(10 * (attempt + 1))  # transient device errors
    global LAST_RESULTS
    LAST_RESULTS = res
    z = np.zeros((T_used, EMBED_DIM), np.float32)
    for r in res.results:
        zt = np.asarray(r["z_t"], np.float32)  # [128, 8, T_used]
        z += zt.transpose(1, 0, 2).reshape(EMBED_DIM, T_used).T
    return z[:Lr]


# ---------------------------------------------------------------------------
# Top-level kernel
# ---------------------------------------------------------------------------

def kernel(vision_feature, text_embed, attention_mask, params):
    vf = np.asarray(vision_feature, np.float32)
    te = np.asarray(text_embed, np.float32)
    am = np.asarray(attention_mask)
    p = {k: np.asarray(v, np.float32) for k, v in params.items()}

    uniq, rem = _route(vf, te, am)
    Ls = len(uniq)
    sel_feat = vf[uniq]
    rem_feat = vf[rem]
    Lr = rem_feat.shape[0]

    # Selected-token path (degenerate MHAs -> linear projections).
    wv_sa = p["sa_wqkv"][2 * EMBED_DIM:]
    bv_sa = p["sa_bqkv"][2 * EMBED_DIM:]
    x1 = (sel_feat @ wv_sa.T + bv_sa) @ p["sa_wo"].T + p["sa_bo"]
    x1 = _ln(x1 + sel_feat, p["n1_w"], p["n1_b"])
    wv_ca = p["ca_wqkv"][2 * EMBED_DIM:]
    bv_ca = p["ca_bqkv"][2 * EMBED_DIM:]
    ca = (x1 @ wv_ca.T + bv_ca) @ p["ca_wo"].T + p["ca_bo"]  # [Ls, D]

    out = sel_feat.copy()
    if Lr > 0:
        bm = np.argmax(_cos(rem_feat, sel_feat), axis=1)  # [Lr]
        t = _ln(ca[bm] + rem_feat, p["n2_w"], p["n2_b"]).astype(np.float32)

        try:
            z = _device_ffn_z(t, p)  # [Lr, D]
        except Exception as e:
            # Last-resort host fallback (device unavailable): exact math.
            import sys
            print(f"device unavailable ({e!r}); host fallback",
                  file=sys.stderr)
            from scipy.special import erf
            h = t @ p["f1_w"].T + p["f1_b"]
            h = h * 0.5 * (1.0 + erf(h / np.sqrt(2.0)))
            z = (h @ p["f2_w"].T).astype(np.float32)

        x3 = _ln(t + z + p["f2_b"], p["n4_w"], p["n4_b"])
        logit = x3 @ p["sp_w"][0] + p["sp_b"][0]
        score = 1.0 / (1.0 + np.exp(-logit))  # [Lr]

        for si in range(Ls):
            rs = np.where(bm == si)[0]
            if len(rs) == 0:
                continue
            vals = np.concatenate([np.ones(1, np.float32), score[rs]])
            w = np.exp(vals - vals.max())
            w /= w.sum()
            out[si] = w[0] * sel_feat[si] + w[1:] @ rem_feat[rs]

    return out.astype(np.float32)
